# revision 28
# baseline (speedup 1.0000x reference)
"""Trainium2 Bass kernel for the MultiHeadAttention transformer block.

Sharding: 8 cores, core c handles batch b=c//2 and query-row half
(c%2)*1024 .. +1024, all 8 heads.  Each core is fully independent.

Key ideas vs the first-generation kernel:
  - The attention mask is folded into a HOST-side gather: only the
    ~L/2 unmasked keys of each (head, batch) row are shipped, so the
    K/V projections, QK, Exp and AV all shrink ~2x.  Padded key slots
    carry k=0 (score 0, exp 1) but v=0 and a 0 in the ones-column, so
    they contribute to neither the numerator nor the denominator.
  - QK runs as two concurrent 64x128 row-tiles of the PE array (the
    contraction is DK=64): head A streams from partitions 0:64, head
    B from 64:128.
  - K/V projections run as two concurrent 128x64 column-tiles and are
    interleaved into the tail of the previous pair's attention loop so
    the Exp stream (the bottleneck engine) never starves.
  - V is projected channel-major and flipped key-major with the DMA
    XBAR transpose (contiguous scratch + strided DVE re-copy).
  - Softmax denominators ride as a 65th ones-column on the AV
    stationary operand; reciprocals broadcast via a DRAM bounce.
  - The LN0 -> fc -> LN1 tail runs its two 512-query halves as
    independent interleaved chains with double-buffered PSUM.
"""

import sys

if "/opt/trn_rl_repo" not in sys.path:
    sys.path.insert(0, "/opt/trn_rl_repo")

import numpy as np

import concourse.bacc as bacc
import concourse.bass as bass
import concourse.tile as tile
from concourse import mybir
from concourse.bass_utils import run_bass_kernel_spmd

H, D, DK, DV = 8, 512, 64, 64
B, L = 4, 2048
P = 128
LQ = L // 2          # query rows per core
NCORES = 8
EPS = 1e-5
F32 = mybir.dt.float32
BF16 = mybir.dt.bfloat16
AF = mybir.ActivationFunctionType
Alu = mybir.AluOpType

DT = D // P     # 4 d-tiles = 4 head pairs
NPAIR = 4

_CACHE = {}


def _bcast(ap, parts):
    """Partition-broadcast view of a [1, n] DRAM AP for DMA replication."""
    return ap.to_broadcast([parts] + list(ap.shape[1:]))


def _emit(nc, tc, n_pad, debug=False):
    NT = n_pad // P  # gathered-key tiles per head
    NCH = 3          # key chunks for the K/V projections
    CH = n_pad // NCH
    assert CH * NCH == n_pad

    qT = nc.dram_tensor("qT", [P, DT, LQ], BF16, kind="ExternalInput")
    kgT = nc.dram_tensor("kgT", [P, DT, H, n_pad], BF16, kind="ExternalInput")
    vgT = nc.dram_tensor("vgT", [P, DT, H, n_pad], BF16, kind="ExternalInput")
    WqT = nc.dram_tensor("WqT", [P, DT, D], BF16, kind="ExternalInput")
    WkT = nc.dram_tensor("WkT", [P, DT, D], BF16, kind="ExternalInput")
    WvT = nc.dram_tensor("WvT", [P, DT, D], BF16, kind="ExternalInput")
    fcwT = nc.dram_tensor("fcwT", [P, DT, D], BF16, kind="ExternalInput")
    ones_in = nc.dram_tensor("ones_in", [P, H, NT], BF16, kind="ExternalInput")
    vecs = nc.dram_tensor("vecs", [5, P, DT], F32, kind="ExternalInput")
    out = nc.dram_tensor("out", [P, DT, LQ], BF16, kind="ExternalOutput")

    with (
        tc.tile_pool(name="consts", bufs=1) as consts,
        tc.tile_pool(name="persist", bufs=1) as persist,
        tc.tile_pool(name="dramp", bufs=4, space="DRAM") as dramp,
    ):
        # ---- constants ----
        gbT = consts.tile([P, 5, DT], F32)
        for i in range(5):
            nc.scalar.dma_start(out=gbT[:, i, :], in_=vecs[i, :, :])
        eps_t = consts.tile([P, 1], F32)
        nc.vector.memset(eps_t, EPS)
        ones_st = consts.tile([P, 1], BF16)
        nc.vector.memset(ones_st, 1.0)
        ones_r1 = consts.tile([1, P], BF16)
        nc.vector.memset(ones_r1, 1.0)
        warm = consts.tile([P, 512], BF16)
        nc.vector.memset(warm[:, :], 0.0)
        expw = consts.tile([1, 1], F32)   # preload the Exp table set
        nc.scalar.activation(out=expw, in_=eps_t[0:1, 0:1], func=AF.Exp,
                             bias=eps_t[0:1, :], scale=1.0)
        ones_g = consts.tile([P, H, NT], BF16)
        nc.scalar.dma_start(out=ones_g, in_=ones_in[:, :, :])
        fcwT_s = consts.tile([P, DT, D], BF16)

        # ---- persistent tiles ----
        qT_s = persist.tile([P, DT, LQ], BF16, tag="qT")   # q^T + residual
        QT_s = persist.tile([P, DT, LQ], BF16, tag="QT")   # projected Q^T
        WqT_s = persist.tile([P, DT, D], BF16, tag="wq")
        WkT_s = persist.tile([P, DT, D], BF16, tag="wk")
        WvT_s = persist.tile([P, DT, D], BF16, tag="wv")
        xbf = persist.tile([P, DT, LQ], BF16, tag="xbf")   # attn + residual
        x2bf = persist.tile([P, DT, LQ], BF16, tag="x2bf")  # its square

        for dt in range(DT):
            nc.scalar.dma_start(out=WqT_s[:, dt, :], in_=WqT[:, dt, :])
            nc.scalar.dma_start(out=WkT_s[:, dt, :], in_=WkT[:, dt, :])
            nc.scalar.dma_start(out=WvT_s[:, dt, :], in_=WvT[:, dt, :])
            nc.scalar.dma_start(out=qT_s[:, dt, :], in_=qT[:, dt, :])
        nc.scalar.dma_start(out=fcwT_s, in_=fcwT[:, :, :])

        # =================== attention (scoped pools) ===================
        with (
            tc.tile_pool(name="kv_in", bufs=2) as kv_in,
            tc.tile_pool(name="kv_proj", bufs=2) as kv_proj,
            tc.tile_pool(name="ptp", bufs=3) as ptp,
            tc.tile_pool(name="stgp", bufs=2) as stgp,
            tc.tile_pool(name="ps", bufs=1, space="PSUM") as ps,
        ):
            # HAM warm-up fodder
            for w in range(8):
                wps = ps.tile([P, 512], F32, tag="qkA", name=f"warm{w}")
                nc.tensor.matmul(wps[:, :], warm[:, 0:P], warm[:, :],
                                 start=True, stop=True)

            # chunked gathered k/v loads: ch-major so the first
            # projection chunk can start after ~1/3 of the data
            kg_tiles, vg_tiles = [], []
            for p in range(NPAIR):
                kg = kv_in.tile([P, DT, 2, n_pad], BF16, tag="kg",
                                name=f"kg{p}")
                vg = kv_in.tile([P, DT, 2, n_pad], BF16, tag="vg",
                                name=f"vg{p}")
                for ch in range(NCH):
                    sl = slice(ch * CH, (ch + 1) * CH)
                    for dt in range(DT):
                        nc.sync.dma_start(out=kg[:, dt, :, sl],
                                          in_=kgT[:, dt, 2 * p:2 * p + 2, sl])
                    for dt in range(DT):
                        nc.sync.dma_start(out=vg[:, dt, :, sl],
                                          in_=vgT[:, dt, 2 * p:2 * p + 2, sl])
                kg_tiles.append(kg)
                vg_tiles.append(vg)

            KT_tiles = [None] * NPAIR
            Vs_tiles = [None] * NPAIR

            def emit_kproj_chunk(p, ch):
                """Column-tiled K projection for key chunk ch of pair p."""
                kg = kg_tiles[p]
                hA, hB = 2 * p, 2 * p + 1
                if ch == 0:
                    KT_tiles[p] = kv_proj.tile([P, n_pad], BF16, tag="KT",
                                               name=f"KT{p}")
                KT_s = KT_tiles[p]
                sl = slice(ch * CH, (ch + 1) * CH)
                psK = ps.tile([P, CH], F32, tag="qkB", name=f"psK{p}_{ch}")
                for dt in range(DT):
                    nc.tensor.matmul(psK[0:64, :],
                                     WkT_s[:, dt, hA * 64:hA * 64 + 64],
                                     kg[:, dt, 0, sl],
                                     start=(dt == 0), stop=(dt == DT - 1))
                    nc.tensor.matmul(psK[64:128, :],
                                     WkT_s[:, dt, hB * 64:hB * 64 + 64],
                                     kg[:, dt, 1, sl],
                                     start=(dt == 0), stop=(dt == DT - 1))
                nc.vector.tensor_copy(KT_s[:, sl], psK[:, :])

            def emit_vproj_chunk(p, ch, VTp_box):
                vg = vg_tiles[p]
                hA, hB = 2 * p, 2 * p + 1
                if ch == 0:
                    VTp_box.append(kv_proj.tile([P, n_pad], BF16, tag="VT",
                                                name=f"VT{p}"))
                VTp = VTp_box[0]
                sl = slice(ch * CH, (ch + 1) * CH)
                psV = ps.tile([P, CH], F32, tag="qkA", name=f"psV{p}_{ch}")
                for dt in range(DT):
                    nc.tensor.matmul(psV[0:64, :],
                                     WvT_s[:, dt, hA * 64:hA * 64 + 64],
                                     vg[:, dt, 0, sl],
                                     start=(dt == 0), stop=(dt == DT - 1))
                    nc.tensor.matmul(psV[64:128, :],
                                     WvT_s[:, dt, hB * 64:hB * 64 + 64],
                                     vg[:, dt, 1, sl],
                                     start=(dt == 0), stop=(dt == DT - 1))
                nc.vector.tensor_copy(VTp[:, sl], psV[:, :])

            def emit_vflip(p, VTp_box):
                """Transpose V to key-major and attach the ones column."""
                VTp = VTp_box[0]
                hA, hB = 2 * p, 2 * p + 1
                V_s = kv_proj.tile([P, NT, 2, DV + 1], BF16, tag="Vs",
                                   name=f"Vs{p}")
                Vs_tiles[p] = V_s
                VAt = kv_proj.tile([P, NT, DV], BF16, tag="Vt",
                                   name=f"VAt{p}")
                VBt = kv_proj.tile([P, NT, DV], BF16, tag="Vt",
                                   name=f"VBt{p}")
                nc.scalar.dma_start(out=VAt[:, :, :], in_=VTp[0:64, :],
                                    transpose=True)
                nc.scalar.dma_start(out=VBt[:, :, :], in_=VTp[64:128, :],
                                    transpose=True)
                nc.vector.tensor_copy(V_s[:, :, 0, 0:DV], VAt[:, :, :])
                nc.vector.tensor_copy(V_s[:, :, 1, 0:DV], VBt[:, :, :])
                nc.vector.tensor_copy(
                    V_s[:, :, 0, DV:DV + 1],
                    ones_g[:, hA, :].rearrange("p (n o) -> p n o", o=1))
                nc.vector.tensor_copy(
                    V_s[:, :, 1, DV:DV + 1],
                    ones_g[:, hB, :].rearrange("p (n o) -> p n o", o=1))

            def emit_qproj(p):
                psQ = ps.tile([P, LQ], F32, tag="qkA", name=f"psQ{p}")
                for dt in range(DT):
                    for jb in range(2):
                        nc.tensor.matmul(psQ[:, jb * 512:(jb + 1) * 512],
                                         WqT_s[:, dt, p * P:(p + 1) * P],
                                         qT_s[:, dt, jb * 512:(jb + 1) * 512],
                                         start=(dt == 0), stop=(dt == DT - 1))
                nc.vector.tensor_copy(QT_s[:, p, :], psQ[:, :])

            def emit_proj(p):
                """Full projection set for pair p."""
                vbox = []
                emit_qproj(p)
                for ch in range(NCH):
                    emit_kproj_chunk(p, ch)
                for ch in range(NCH):
                    emit_vproj_chunk(p, ch, vbox)
                emit_vflip(p, vbox)

            emit_proj(0)

            for p in range(NPAIR):
                KT_s = KT_tiles[p]
                V_s = Vs_tiles[p]
                vbox_next = []
                avA = ps.tile([DV + 1, LQ], F32, tag="av", bufs=2,
                              name=f"avA{p}")
                avB = ps.tile([DV + 1, LQ], F32, tag="av", bufs=2,
                              name=f"avB{p}")
                for m in range(NT):
                    msl = slice(m * P, (m + 1) * P)
                    psA = ps.tile([P, LQ], F32, tag="qkA", name=f"psA{p}_{m}")
                    psB = ps.tile([P, LQ], F32, tag="qkB", name=f"psB{p}_{m}")
                    for jb in range(2):
                        jsl = slice(jb * 512, (jb + 1) * 512)
                        nc.tensor.matmul(psA[:, jsl], KT_s[0:64, msl],
                                         QT_s[0:64, p, jsl],
                                         start=True, stop=True)
                        nc.tensor.matmul(psB[:, jsl], KT_s[64:128, msl],
                                         QT_s[64:128, p, jsl],
                                         start=True, stop=True)
                    ptA = ptp.tile([P, LQ], BF16, tag="pt",
                                   name=f"ptA{p}_{m}")
                    nc.scalar.activation(out=ptA, in_=psA[:, :], func=AF.Exp,
                                         scale=1.0 / 8.0)
                    ptB = ptp.tile([P, LQ], BF16, tag="pt",
                                   name=f"ptB{p}_{m}")
                    nc.scalar.activation(out=ptB, in_=psB[:, :], func=AF.Exp,
                                         scale=1.0 / 8.0)
                    for jb in range(2):
                        jsl = slice(jb * 512, (jb + 1) * 512)
                        nc.tensor.matmul(avA[:, jsl], V_s[:, m, 0, :],
                                         ptA[:, jsl],
                                         start=(m == 0), stop=(m == NT - 1))
                        nc.tensor.matmul(avB[:, jsl], V_s[:, m, 1, :],
                                         ptB[:, jsl],
                                         start=(m == 0), stop=(m == NT - 1))
                    # interleave the next pair's projections into the
                    # tail of this m-loop so Exp never starves
                    if p + 1 < NPAIR:
                        if m == NT - 3:
                            for ch in range(NCH):
                                emit_kproj_chunk(p + 1, ch)
                        elif m == NT - 2:
                            for ch in range(NCH):
                                emit_vproj_chunk(p + 1, ch, vbox_next)

                if p + 1 < NPAIR:
                    emit_qproj(p + 1)
                    emit_vflip(p + 1, vbox_next)

                # ---- divide by the softmax denominator, write xbf ----
                for hh, av in ((0, avA), (1, avB)):
                    stg = stgp.tile([DV + 1, LQ], F32, tag="stg")
                    nc.vector.tensor_copy(stg, av[:, :])
                    rcd = dramp.tile([1, LQ], F32, tag="rcd",
                                     name=f"rcd{p}_{hh}")
                    nc.scalar.dma_start(out=rcd, in_=stg[DV:DV + 1, :])
                    bcs = stgp.tile([DV, LQ], F32, tag="bcs")
                    nc.gpsimd.dma_start(out=bcs, in_=_bcast(rcd[0:1, :], DV))
                    nc.vector.reciprocal_approx_fast(out=bcs, in_=bcs)
                    nc.vector.tensor_mul(xbf[hh * DV:(hh + 1) * DV, p, :],
                                         stg[0:DV, :], bcs[:, :])
                nc.gpsimd.tensor_add(xbf[:, p, :], xbf[:, p, :],
                                     qT_s[:, p, :])
                nc.gpsimd.tensor_mul(x2bf[:, p, :], xbf[:, p, :],
                                     xbf[:, p, :])

        # ============ LN0 -> fc -> LN1 tail (transposed, bf16) ============
        with (
            tc.tile_pool(name="chain", bufs=2) as chain,
            tc.tile_pool(name="bcB", bufs=4) as bcB,
            tc.tile_pool(name="psL", bufs=1, space="PSUM") as psL,
        ):
            # reuse dead persistent buffers for the tail
            outT = persist.tile([P, DT, LQ], BF16, tag="qT", name="outT")
            y2bf = persist.tile([P, DT, LQ], BF16, tag="QT", name="y2bf")
            ybf = persist.tile([P, DT, LQ], BF16, tag="x2bf", name="ybf")

            def ln_half(xb, x2b, g_idx, b_idx, nh, label, final_out=None):
                sl = slice(nh * 512, (nh + 1) * 512)
                s1 = psL.tile([1, 512], F32, tag="s1", bufs=2,
                              name=f"s1{label}{nh}")
                s2 = psL.tile([1, 512], F32, tag="s2", bufs=2,
                              name=f"s2{label}{nh}")
                for kt in range(DT):
                    nc.tensor.matmul(s1[:, :], ones_st[:, :], xb[:, kt, sl],
                                     start=(kt == 0), stop=(kt == DT - 1))
                    nc.tensor.matmul(s2[:, :], ones_st[:, :], x2b[:, kt, sl],
                                     start=(kt == 0), stop=(kt == DT - 1))
                mu = chain.tile([1, 512], F32, tag="mu")
                nc.vector.tensor_scalar_mul(mu, s1[:, :], 1.0 / D)
                var = chain.tile([1, 512], F32, tag="var")
                nc.vector.tensor_mul(var, mu, mu)
                msq = chain.tile([1, 512], F32, tag="msq")
                nc.vector.tensor_scalar_mul(msq, s2[:, :], 1.0 / D)
                nc.vector.tensor_sub(var, msq, var)
                nc.scalar.activation(out=var, in_=var, func=AF.Sqrt,
                                     bias=eps_t[0:1, :])
                rstd = chain.tile([1, 512], F32, tag="rstd")
                nc.vector.reciprocal_approx_fast(out=rstd, in_=var)
                mrb = chain.tile([1, 2, 512], BF16, tag="mrb")
                nc.vector.tensor_copy(mrb[:, 0, :], mu[0:1, :])
                nc.vector.tensor_copy(mrb[:, 1, :], rstd[0:1, :])
                mu_b = psL.tile([P, 512], F32, tag="bc", bufs=2,
                                name=f"mb{label}{nh}")
                nc.tensor.matmul(mu_b[:, :], ones_r1[:, :], mrb[:, 0, :],
                                 start=True, stop=True)
                rstd_b = psL.tile([P, 512], F32, tag="bc", bufs=2,
                                  name=f"rb{label}{nh}")
                nc.tensor.matmul(rstd_b[:, :], ones_r1[:, :], mrb[:, 1, :],
                                 start=True, stop=True)
                mu_bb = bcB.tile([P, 512], BF16, tag="bc",
                                 name=f"mbb{label}{nh}")
                nc.vector.tensor_copy(mu_bb, mu_b[:, :])
                rstd_bb = bcB.tile([P, 512], BF16, tag="bc",
                                   name=f"rbb{label}{nh}")
                nc.vector.tensor_copy(rstd_bb, rstd_b[:, :])
                for kt in range(DT):
                    nc.vector.tensor_sub(xb[:, kt, sl], xb[:, kt, sl],
                                         mu_bb[:, :])
                    nc.vector.tensor_mul(xb[:, kt, sl], xb[:, kt, sl],
                                         rstd_bb[:, :])
                    tgt = final_out if final_out is not None else xb
                    nc.vector.tensor_scalar(
                        out=tgt[:, kt, sl], in0=xb[:, kt, sl],
                        scalar1=gbT[:, g_idx, kt:kt + 1],
                        scalar2=gbT[:, b_idx, kt:kt + 1],
                        op0=Alu.mult, op1=Alu.add)

            for nh in range(2):
                ln_half(xbf, x2bf, 0, 1, nh, "a")
            for nh in range(2):
                sl = slice(nh * 512, (nh + 1) * 512)
                for m in range(DT):
                    psf = psL.tile([P, 512], F32, tag="fc", bufs=2,
                                   name=f"fc{m}_{nh}")
                    for dt in range(DT):
                        nc.tensor.matmul(psf[:, :],
                                         fcwT_s[:, dt, m * P:(m + 1) * P],
                                         xbf[:, dt, sl],
                                         start=(dt == 0), stop=(dt == DT - 1))
                    nc.scalar.activation(
                        out=ybf[:, m, sl], in_=psf[:, :], func=AF.Identity,
                        bias=gbT[:, 4, m:m + 1])
                    nc.gpsimd.tensor_add(ybf[:, m, sl], ybf[:, m, sl],
                                         xbf[:, m, sl])
                    nc.vector.tensor_mul(y2bf[:, m, sl], ybf[:, m, sl],
                                         ybf[:, m, sl])
                ln_half(ybf, y2bf, 2, 3, nh, "b", final_out=outT)
                for kt in range(DT):
                    nc.scalar.dma_start(out=out[:, kt, sl],
                                        in_=outT[:, kt, sl])


def _build(n_pad):
    key = ("nc", n_pad)
    if key in _CACHE:
        return _CACHE[key]
    nc = bacc.Bacc(None, target_bir_lowering=False, debug=False)
    with tile.TileContext(nc) as tc:
        _emit(nc, tc, n_pad)
    nc.compile()
    _CACHE[key] = nc
    return nc


def _ptile(a):
    """[n, m] -> transpose -> [m(=tiles*128), n] -> [128, tiles, n]"""
    t = np.asarray(a, np.float32).T
    return np.ascontiguousarray(
        t.reshape(t.shape[0] // P, P, t.shape[1]).transpose(1, 0, 2))


def _prep_in_maps(q, k, v, mask, Wq, Wk, Wv, fc_w, fc_b, g0, b0, g1, b1):
    q = np.asarray(q, np.float32)
    k = np.asarray(k, np.float32)
    v = np.asarray(v, np.float32)
    mask = np.asarray(mask)
    bf = mybir.dt.np(BF16)

    idxs = [np.nonzero(mask[n])[0] for n in range(H * B)]
    n_max = max(len(ix) for ix in idxs)
    n_pad = -((-n_max) // 384) * 384  # divisible by both 128 and 3
    NT = n_pad // P

    WqTh = _ptile(Wq).astype(bf)
    WkTh = _ptile(Wk).astype(bf)
    WvTh = _ptile(Wv).astype(bf)
    fcwTh = _ptile(fc_w).astype(bf)
    vecs = np.stack([np.asarray(x, np.float32).reshape(DT, P).T
                     for x in (g0, b0, g1, b1, fc_b)])
    vecs = np.ascontiguousarray(vecs)

    in_maps = []
    for c in range(NCORES):
        b = c // 2
        r0 = (c % 2) * LQ
        qTb = _ptile(q[b][r0:r0 + LQ]).astype(bf)
        kgT = np.zeros((P, DT, H, n_pad), bf)
        vgT = np.zeros((P, DT, H, n_pad), bf)
        ones = np.zeros((P, H, NT), bf)
        for h in range(H):
            ix = idxs[h * B + b]
            n = len(ix)
            kg = np.zeros((D, n_pad), np.float32)
            vg = np.zeros((D, n_pad), np.float32)
            kg[:, :n] = k[b][ix].T
            vg[:, :n] = v[b][ix].T
            kgT[:, :, h, :] = kg.reshape(DT, P, n_pad).transpose(1, 0, 2)
            vgT[:, :, h, :] = vg.reshape(DT, P, n_pad).transpose(1, 0, 2)
            o = np.zeros(n_pad, np.float32)
            o[:n] = 1.0
            ones[:, h, :] = o.reshape(NT, P).T
        in_maps.append({
            "qT": qTb, "kgT": kgT, "vgT": vgT,
            "WqT": WqTh, "WkT": WkTh, "WvT": WvTh, "fcwT": fcwTh,
            "ones_in": np.ascontiguousarray(ones), "vecs": vecs,
        })
    return in_maps, n_pad


def kernel(q, k, v, mask, Wq, Wk, Wv, fc_w, fc_b, g0, b0, g1, b1):
    in_maps, n_pad = _prep_in_maps(q, k, v, mask, Wq, Wk, Wv, fc_w, fc_b,
                                   g0, b0, g1, b1)
    nc = _build(n_pad)
    res = run_bass_kernel_spmd(nc, in_maps, core_ids=list(range(NCORES)))
    outf = np.empty((B, L, D), np.float32)
    for c in range(NCORES):
        b = c // 2
        r0 = (c % 2) * LQ
        o = np.asarray(res.results[c]["out"], np.float32)  # [128, DT, LQ]
        outf[b, r0:r0 + LQ, :] = o.transpose(2, 1, 0).reshape(LQ, D)
    return outf
